# revision 4
# baseline (speedup 1.0000x reference)
"""Grouped-experts MoE (SwiGLU) Bass kernel for Trainium2, 8 NeuronCores.

Expert-parallel: core c owns experts [8c, 8c+8). Tokens are pre-grouped by
expert in the input, so routing is host-side slicing. All device matmuls run
in transposed-token space so every operand streams in its natural layout:

  gateT[i, t] = sum_k G[k, i] * xT[k, t]      (lhsT = G tile, rhs = xT tile)
  hT = silu(gateT) * upT                       (elementwise, [inter, tok])
  outT[m, t] = sum_ki D[ki, m] * hT[ki, t]     (lhsT = D tile, rhs = hT tile)

v2 layout strategy: the host pre-swizzles every operand into the exact
column order the device consumes, so each SBUF tile is filled by ONE large
contiguous DMA:
  - x:   one tile per (segment, chunk): [128, 16k x csz]
  - G/U: one tile per (segment, i-pair): [128, 16k x 2i x 128]
  - D:   one tile per (segment, m-quartet): [128, 6ki x 4m x 128]
Fine tile granularity means tiles die early in each phase, freeing pool
slots so next-segment DMAs prefetch ~a full phase ahead (no PE stalls at
segment boundaries). Output is written bf16 (halves store traffic).
A burst of dummy matmuls on zeroed SBUF warms the PE HAM clock-gate during
the initial DMA ramp so real matmuls never run at the cold 1.2 GHz rate.
"""

import numpy as np
import ml_dtypes

NUM_EXPERTS = 64
HID = 2048
INTER = 768
N_CORES = 8
EPC = NUM_EXPERTS // N_CORES  # experts per core
KT = HID // 128    # 16 k-tiles over hidden
IT = INTER // 128  # 6 tiles over intermediate
NPAIR = IT // 2    # 3 i-pairs (gate/up weight tile granularity)
NQUAD = KT // 4    # 4 m-quartets (down weight tile granularity)
CHUNK = 512        # max moving-operand free dim per matmul
N_WARM = 24        # dummy matmuls to warm the PE clock gate

BF16_NP = ml_dtypes.bfloat16

_cache = {}


def _chunks(p):
    """Balanced split into ceil(p/CHUNK) near-equal chunks."""
    if p <= 0:
        return []
    nch = -(-p // CHUNK)
    base, rem = divmod(p, nch)
    out = []
    n0 = 0
    for i in range(nch):
        sz = base + (1 if i < rem else 0)
        out.append((n0, sz))
        n0 += sz
    return out


def _layout(padded):
    """Per-segment chunk lists and x-block column offsets in xt2."""
    segs = []
    xoff = 0
    col0 = 0
    for j, p in enumerate(padded):
        ch = _chunks(p)
        xoffs = []
        for (n0, csz) in ch:
            xoffs.append(xoff)
            xoff += KT * csz
        segs.append((j, col0, ch, xoffs))
        col0 += p
    return segs, xoff


def _build(padded):
    import concourse.bacc as bacc
    import concourse.mybir as mybir
    import concourse.tile as tile

    BF16 = mybir.dt.bfloat16
    F32 = mybir.dt.float32
    SILU = mybir.ActivationFunctionType.Silu

    ptot = int(sum(padded))
    segs, xcols = _layout(padded)
    max_csz = max(csz for (_, _, ch, _) in segs for (_, csz) in ch)
    max_nch = max(len(ch) for (_, _, ch, _) in segs)
    h_live = IT * max_nch  # all h tiles of one segment live through its down phase

    nc = bacc.Bacc("TRN2", target_bir_lowering=False, debug=False,
                   num_devices=N_CORES)

    xt2 = nc.dram_tensor("xt2", [128, xcols], BF16, kind="ExternalInput")
    gw2 = nc.dram_tensor("gw2", [128, EPC * NPAIR * KT * 256], BF16,
                         kind="ExternalInput")
    uw2 = nc.dram_tensor("uw2", [128, EPC * NPAIR * KT * 256], BF16,
                         kind="ExternalInput")
    dw2 = nc.dram_tensor("dw2", [128, EPC * NQUAD * IT * 512], BF16,
                         kind="ExternalInput")
    yt = nc.dram_tensor("yt", [HID, ptot], BF16, kind="ExternalOutput")

    WCOLS = KT * 256   # columns per (slot, pair) G/U block
    DCOLS = IT * 512   # columns per (slot, quartet) D block

    with tile.TileContext(nc) as tc:
        with (
            tc.tile_pool(name="dmp", bufs=1) as dmp,
            tc.tile_pool(name="xp", bufs=max_nch + 2) as xp,
            tc.tile_pool(name="wp", bufs=6) as wp,
            tc.tile_pool(name="dp", bufs=3) as dp,
            tc.tile_pool(name="hp", bufs=h_live + 2) as hp,
            tc.tile_pool(name="sp", bufs=3) as sp,
            tc.tile_pool(name="op", bufs=6) as op,
            tc.tile_pool(name="psg", bufs=3, space="PSUM") as psg,
            tc.tile_pool(name="psu", bufs=3, space="PSUM") as psu,
            tc.tile_pool(name="psd", bufs=2, space="PSUM") as psd,
        ):
            # ---- PE warm-up: dummy matmuls on zeroed SBUF during DMA ramp
            dmy = dmp.tile([128, 640], BF16, tag="dmy")
            nc.vector.memset(dmy[:], 0.0)
            pwarm = psd.tile([128, 512], F32, tag="pd")
            for _ in range(N_WARM):
                nc.tensor.matmul(pwarm[:], dmy[:, 0:128], dmy[:, 128:640],
                                 start=True, stop=True)

            for (slot, col0, ch, xoffs) in segs:
                nch = len(ch)

                # ---- loads (consumption order; each one big contiguous DMA)
                xts = []
                for ci, (n0, csz) in enumerate(ch):
                    t = xp.tile([128, KT * max_csz], BF16, tag="x")
                    nc.sync.dma_start(
                        t[:, :KT * csz],
                        xt2[:, xoffs[ci]:xoffs[ci] + KT * csz])
                    xts.append(t)
                    if ci == 0:
                        gt, ut = [], []
                        for p in range(NPAIR):
                            g = wp.tile([128, WCOLS], BF16, tag="w")
                            base = (slot * NPAIR + p) * WCOLS
                            nc.sync.dma_start(g[:], gw2[:, base:base + WCOLS])
                            gt.append(g)
                            u = wp.tile([128, WCOLS], BF16, tag="w")
                            nc.sync.dma_start(u[:], uw2[:, base:base + WCOLS])
                            ut.append(u)
                dt = []
                for q in range(NQUAD):
                    d = dp.tile([128, DCOLS], BF16, tag="d")
                    base = (slot * NQUAD + q) * DCOLS
                    nc.sync.dma_start(d[:], dw2[:, base:base + DCOLS])
                    dt.append(d)

                # ---- gate/up phase ----
                h = {}
                for i in range(IT):
                    p, io = divmod(i, 2)
                    for ci, (n0, csz) in enumerate(ch):
                        pg = psg.tile([128, csz], F32, tag="pg")
                        for k in range(KT):
                            w0 = k * 256 + io * 128
                            nc.tensor.matmul(pg[:],
                                             gt[p][:, w0:w0 + 128],
                                             xts[ci][:, k * csz:(k + 1) * csz],
                                             start=(k == 0), stop=(k == KT - 1))
                        pu = psu.tile([128, csz], F32, tag="pu")
                        for k in range(KT):
                            w0 = k * 256 + io * 128
                            nc.tensor.matmul(pu[:],
                                             ut[p][:, w0:w0 + 128],
                                             xts[ci][:, k * csz:(k + 1) * csz],
                                             start=(k == 0), stop=(k == KT - 1))
                        st = sp.tile([128, csz], F32, tag="s")
                        nc.scalar.activation(st[:], pg[:], SILU)
                        ht = hp.tile([128, csz], BF16, tag="h")
                        nc.vector.tensor_mul(ht[:], st[:], pu[:])
                        h[(i, ci)] = ht

                # ---- down phase ----
                for m in range(KT):
                    q, mq = divmod(m, 4)
                    for ci, (n0, csz) in enumerate(ch):
                        pd = psd.tile([128, csz], F32, tag="pd")
                        for ki in range(IT):
                            w0 = ki * 512 + mq * 128
                            nc.tensor.matmul(pd[:],
                                             dt[q][:, w0:w0 + 128],
                                             h[(ki, ci)][:],
                                             start=(ki == 0),
                                             stop=(ki == IT - 1))
                        ot = op.tile([128, csz], BF16, tag="o")
                        nc.vector.tensor_copy(ot[:], pd[:])
                        nc.gpsimd.dma_start(
                            yt[m * 128:(m + 1) * 128,
                               col0 + n0:col0 + n0 + csz], ot[:])

    nc.compile()
    return nc, ptot


def _get_program(padded):
    key = tuple(padded)
    if key not in _cache:
        _cache[key] = _build(padded)
    return _cache[key]


def _swizzle_x(xtc, segs):
    """[HID, ptot] -> [128, 16k x csz] blocks in device consumption order."""
    blocks = []
    for (_, col0, ch, _) in segs:
        for (n0, csz) in ch:
            blk = xtc[:, col0 + n0:col0 + n0 + csz]
            blocks.append(blk.reshape(KT, 128, csz).transpose(1, 0, 2)
                          .reshape(128, KT * csz))
    return np.ascontiguousarray(np.concatenate(blocks, axis=1))


def _swizzle_gu(w):
    """[EPC, HID, INTER] -> [128, EPC*3pair*(16k x 2io x 128)]."""
    # [e, k*128+p, (2pr+io)*128+c] -> cols (e, pr, k, io, c)
    arr = w.reshape(EPC, KT, 128, NPAIR, 2, 128)
    arr = arr.transpose(2, 0, 3, 1, 4, 5)  # [p, e, pr, k, io, c]
    return np.ascontiguousarray(arr.reshape(128, EPC * NPAIR * KT * 256))


def _swizzle_d(w):
    """[EPC, INTER, HID] -> [128, EPC*4quad*(6ki x 4mq x 128)]."""
    # [e, ki*128+p, (4q+mq)*128+c] -> cols (e, q, ki, mq, c)
    arr = w.reshape(EPC, IT, 128, NQUAD, 4, 128)
    arr = arr.transpose(2, 0, 3, 1, 4, 5)  # [p, e, q, ki, mq, c]
    return np.ascontiguousarray(arr.reshape(128, EPC * NQUAD * IT * 512))


def _invoke(x, gate_proj, up_proj, down_proj, num_tokens_per_expert,
            trace=False, trace_kwargs=None):
    from concourse.bass_utils import run_bass_kernel_spmd

    x = np.asarray(x)
    counts = np.asarray(num_tokens_per_expert).astype(np.int64)
    assert counts.shape == (NUM_EXPERTS,)
    starts = np.zeros(NUM_EXPERTS + 1, dtype=np.int64)
    np.cumsum(counts, out=starts[1:])

    # per-slot padded counts (max over cores) -> one SPMD program
    cmat = counts.reshape(N_CORES, EPC)
    padded = [int(cmat[:, j].max()) for j in range(EPC)]
    offs = np.zeros(EPC + 1, dtype=np.int64)
    np.cumsum(np.asarray(padded), out=offs[1:])

    nc, ptot = _get_program(padded)
    assert ptot == int(offs[-1])
    segs, _ = _layout(padded)

    gb = np.asarray(gate_proj).astype(BF16_NP)
    ub = np.asarray(up_proj).astype(BF16_NP)
    db = np.asarray(down_proj).astype(BF16_NP)

    in_maps = []
    for c in range(N_CORES):
        xtc = np.zeros((HID, ptot), dtype=BF16_NP)
        for j in range(EPC):
            e = c * EPC + j
            cnt = int(counts[e])
            if cnt:
                xtc[:, int(offs[j]):int(offs[j]) + cnt] = \
                    x[int(starts[e]):int(starts[e]) + cnt].astype(BF16_NP).T
        in_maps.append({
            "xt2": _swizzle_x(xtc, segs),
            "gw2": _swizzle_gu(gb[c * EPC:(c + 1) * EPC]),
            "uw2": _swizzle_gu(ub[c * EPC:(c + 1) * EPC]),
            "dw2": _swizzle_d(db[c * EPC:(c + 1) * EPC]),
        })

    res = run_bass_kernel_spmd(nc, in_maps, list(range(N_CORES)),
                               trace=trace, **(trace_kwargs or {}))

    out = np.empty((int(starts[-1]), HID), dtype=np.float32)
    for c in range(N_CORES):
        ytc = res.results[c]["yt"]
        for j in range(EPC):
            e = c * EPC + j
            cnt = int(counts[e])
            if cnt:
                out[int(starts[e]):int(starts[e]) + cnt] = \
                    ytc[:, int(offs[j]):int(offs[j]) + cnt].T.astype(np.float32)
    return out, res


def kernel(x, gate_proj, up_proj, down_proj, num_tokens_per_expert):
    out, _ = _invoke(x, gate_proj, up_proj, down_proj, num_tokens_per_expert)
    return out


# revision 8
# speedup vs baseline: 1.0098x; 1.0098x over previous
"""Grouped-experts MoE (SwiGLU) Bass kernel for Trainium2, 8 NeuronCores.

Expert-parallel: core c owns experts [8c, 8c+8). Tokens are pre-grouped by
expert in the input, so routing is host-side slicing. All device matmuls run
in transposed-token space so every operand streams in its natural layout:

  gateT[i, t] = sum_k G[k, i] * xT[k, t]      (lhsT = G tile, rhs = xT tile)
  hT = silu(gateT) * upT                       (elementwise, [inter, tok])
  outT[m, t] = sum_ki D[ki, m] * hT[ki, t]     (lhsT = D tile, rhs = hT tile)

v2 layout strategy: the host pre-swizzles every operand into the exact
column order the device consumes, so each SBUF tile is filled by ONE large
contiguous DMA:
  - x:   one tile per (segment, chunk): [128, 16k x csz]
  - G/U: one tile per (segment, i-pair): [128, 16k x 2i x 128]
  - D:   one tile per (segment, m-quartet): [128, 6ki x 4m x 128]
Fine tile granularity means tiles die early in each phase, freeing pool
slots so next-segment DMAs prefetch ~a full phase ahead (no PE stalls at
segment boundaries). Output is written bf16 (halves store traffic).
A burst of dummy matmuls on zeroed SBUF warms the PE HAM clock-gate during
the initial DMA ramp so real matmuls never run at the cold 1.2 GHz rate.
"""

import numpy as np
import ml_dtypes

NUM_EXPERTS = 64
HID = 2048
INTER = 768
N_CORES = 8
EPC = NUM_EXPERTS // N_CORES  # experts per core
KT = HID // 128    # 16 k-tiles over hidden
IT = INTER // 128  # 6 tiles over intermediate
NPAIR = IT // 2    # 3 i-pairs (gate/up weight tile granularity)
NQUAD = KT // 4    # 4 m-quartets (down weight tile granularity)
CHUNK = 512        # max moving-operand free dim per matmul
N_WARM = 24        # dummy matmuls to warm the PE clock gate

BF16_NP = ml_dtypes.bfloat16

_cache = {}


def _chunks(p):
    """Balanced split into ceil(p/CHUNK) near-equal chunks."""
    if p <= 0:
        return []
    nch = -(-p // CHUNK)
    base, rem = divmod(p, nch)
    out = []
    n0 = 0
    for i in range(nch):
        sz = base + (1 if i < rem else 0)
        out.append((n0, sz))
        n0 += sz
    return out


def _layout(padded):
    """Per-segment chunk lists and x-block column offsets in xt2."""
    segs = []
    xoff = 0
    col0 = 0
    for j, p in enumerate(padded):
        ch = _chunks(p)
        xoffs = []
        for (n0, csz) in ch:
            xoffs.append(xoff)
            xoff += KT * csz
        segs.append((j, col0, ch, xoffs))
        col0 += p
    return segs, xoff


def _build(padded):
    import concourse.bacc as bacc
    import concourse.mybir as mybir
    import concourse.tile as tile
    from concourse.tile import add_dep_helper

    BF16 = mybir.dt.bfloat16
    F32 = mybir.dt.float32
    SILU = mybir.ActivationFunctionType.Silu

    ptot = int(sum(padded))
    segs, xcols = _layout(padded)
    max_csz = max(csz for (_, _, ch, _) in segs for (_, csz) in ch)
    max_nch = max(len(ch) for (_, _, ch, _) in segs)
    h_live = IT * max_nch  # all h tiles of one segment live through its down phase

    nc = bacc.Bacc("TRN2", target_bir_lowering=False, debug=False,
                   num_devices=N_CORES)

    xt2 = nc.dram_tensor("xt2", [128, xcols], BF16, kind="ExternalInput")
    gw2 = nc.dram_tensor("gw2", [128, EPC * NPAIR * KT * 256], BF16,
                         kind="ExternalInput")
    uw2 = nc.dram_tensor("uw2", [128, EPC * NPAIR * KT * 256], BF16,
                         kind="ExternalInput")
    dw2 = nc.dram_tensor("dw2", [128, EPC * NQUAD * IT * 512], BF16,
                         kind="ExternalInput")
    yt = nc.dram_tensor("yt", [HID, ptot], BF16, kind="ExternalOutput")

    WCOLS = KT * 256   # columns per (slot, pair) G/U block
    DCOLS = IT * 512   # columns per (slot, quartet) D block

    with tile.TileContext(nc) as tc:
        with (
            tc.tile_pool(name="dmp", bufs=1) as dmp,
            tc.tile_pool(name="xp", bufs=max_nch + 2) as xp,
            tc.tile_pool(name="wp", bufs=8) as wp,
            tc.tile_pool(name="dp", bufs=3) as dp,
            tc.tile_pool(name="hp", bufs=h_live + 2) as hp,
            tc.tile_pool(name="sp", bufs=3) as sp,
            tc.tile_pool(name="op", bufs=6) as op,
            tc.tile_pool(name="psg", bufs=3, space="PSUM") as psg,
            tc.tile_pool(name="psu", bufs=3, space="PSUM") as psu,
            tc.tile_pool(name="psd", bufs=2, space="PSUM") as psd,
        ):
            # Chain every PE accumulation group onto the previous one
            # (order-only edges): the scheduler otherwise hoists groups whose
            # DMA it models as ready; a mispredict blocks the in-order PE
            # behind a waiting matmul while ready work sits queued.
            prev_mm = [None]

            def group(mms):
                first = None
                for (out_ap, lhsT, rhs, start, stop) in mms:
                    ins = nc.tensor.matmul(out_ap, lhsT, rhs,
                                           start=start, stop=stop)
                    if first is None:
                        first = ins
                        if prev_mm[0] is not None:
                            add_dep_helper(ins.ins, prev_mm[0].ins, sync=False,
                                           reason="pe-order")
                prev_mm[0] = ins

            # ---- PE warm-up: dummy matmuls on zeroed SBUF during DMA ramp
            dmy = dmp.tile([128, 640], BF16, tag="dmy")
            nc.vector.memset(dmy[:], 0.0)
            pwarm = psd.tile([128, 512], F32, tag="pd")
            for _ in range(N_WARM):
                group([(pwarm[:], dmy[:, 0:128], dmy[:, 128:640], True, True)])

            for si, (slot, col0, ch, xoffs) in enumerate(segs):
                nch = len(ch)
                halves = 2 if si == 0 else 1  # split first loads -> faster ramp

                # ---- loads (consumption order; big contiguous DMAs)
                # x on the ACT HWDGE ring, weights on the SP ring: two
                # independent FIFOs so x prefetch never queues behind weights.
                xts = []
                for ci, (n0, csz) in enumerate(ch):
                    t = xp.tile([128, KT * max_csz], BF16, tag="x")
                    nh = halves if ci == 0 else 1
                    step = KT * csz // nh
                    for hh in range(nh):
                        nc.scalar.dma_start(
                            t[:, hh * step:(hh + 1) * step],
                            xt2[:, xoffs[ci] + hh * step:
                                 xoffs[ci] + (hh + 1) * step])
                    xts.append(t)
                    if ci == 0:
                        gt, ut = [], []
                        for p in range(NPAIR):
                            nh = halves if p == 0 else 1
                            wstep = WCOLS // nh
                            g = wp.tile([128, WCOLS], BF16, tag="w")
                            base = (slot * NPAIR + p) * WCOLS
                            for hh in range(nh):
                                nc.sync.dma_start(
                                    g[:, hh * wstep:(hh + 1) * wstep],
                                    gw2[:, base + hh * wstep:
                                        base + (hh + 1) * wstep])
                            gt.append(g)
                            u = wp.tile([128, WCOLS], BF16, tag="w")
                            for hh in range(nh):
                                nc.sync.dma_start(
                                    u[:, hh * wstep:(hh + 1) * wstep],
                                    uw2[:, base + hh * wstep:
                                        base + (hh + 1) * wstep])
                            ut.append(u)
                dt = []
                for q in range(NQUAD):
                    d = dp.tile([128, DCOLS], BF16, tag="d")
                    base = (slot * NQUAD + q) * DCOLS
                    nc.sync.dma_start(d[:], dw2[:, base:base + DCOLS])
                    dt.append(d)

                # ---- gate/up phase ----
                h = {}
                for i in range(IT):
                    p, io = divmod(i, 2)
                    for ci, (n0, csz) in enumerate(ch):
                        pg = psg.tile([128, csz], F32, tag="pg")
                        group([(pg[:],
                                gt[p][:, k * 256 + io * 128:
                                      k * 256 + io * 128 + 128],
                                xts[ci][:, k * csz:(k + 1) * csz],
                                k == 0, k == KT - 1) for k in range(KT)])
                        pu = psu.tile([128, csz], F32, tag="pu")
                        group([(pu[:],
                                ut[p][:, k * 256 + io * 128:
                                      k * 256 + io * 128 + 128],
                                xts[ci][:, k * csz:(k + 1) * csz],
                                k == 0, k == KT - 1) for k in range(KT)])
                        st = sp.tile([128, csz], F32, tag="s")
                        nc.scalar.activation(st[:], pg[:], SILU)
                        ht = hp.tile([128, csz], BF16, tag="h")
                        nc.vector.tensor_mul(ht[:], st[:], pu[:])
                        h[(i, ci)] = ht

                # ---- down phase ----
                for m in range(KT):
                    q, mq = divmod(m, 4)
                    for ci, (n0, csz) in enumerate(ch):
                        pd = psd.tile([128, csz], F32, tag="pd")
                        group([(pd[:],
                                dt[q][:, ki * 512 + mq * 128:
                                      ki * 512 + mq * 128 + 128],
                                h[(ki, ci)][:],
                                ki == 0, ki == IT - 1) for ki in range(IT)])
                        ot = op.tile([128, csz], BF16, tag="o")
                        nc.vector.tensor_copy(ot[:], pd[:])
                        nc.gpsimd.dma_start(
                            yt[m * 128:(m + 1) * 128,
                               col0 + n0:col0 + n0 + csz], ot[:])

    nc.compile()
    return nc, ptot


def _get_program(padded):
    key = tuple(padded)
    if key not in _cache:
        _cache[key] = _build(padded)
    return _cache[key]


def _swizzle_x(xtc, segs):
    """[HID, ptot] -> [128, 16k x csz] blocks in device consumption order."""
    blocks = []
    for (_, col0, ch, _) in segs:
        for (n0, csz) in ch:
            blk = xtc[:, col0 + n0:col0 + n0 + csz]
            blocks.append(blk.reshape(KT, 128, csz).transpose(1, 0, 2)
                          .reshape(128, KT * csz))
    return np.ascontiguousarray(np.concatenate(blocks, axis=1))


def _swizzle_gu(w):
    """[EPC, HID, INTER] -> [128, EPC*3pair*(16k x 2io x 128)]."""
    # [e, k*128+p, (2pr+io)*128+c] -> cols (e, pr, k, io, c)
    arr = w.reshape(EPC, KT, 128, NPAIR, 2, 128)
    arr = arr.transpose(2, 0, 3, 1, 4, 5)  # [p, e, pr, k, io, c]
    return np.ascontiguousarray(arr.reshape(128, EPC * NPAIR * KT * 256))


def _swizzle_d(w):
    """[EPC, INTER, HID] -> [128, EPC*4quad*(6ki x 4mq x 128)]."""
    # [e, ki*128+p, (4q+mq)*128+c] -> cols (e, q, ki, mq, c)
    arr = w.reshape(EPC, IT, 128, NQUAD, 4, 128)
    arr = arr.transpose(2, 0, 3, 1, 4, 5)  # [p, e, q, ki, mq, c]
    return np.ascontiguousarray(arr.reshape(128, EPC * NQUAD * IT * 512))


def _invoke(x, gate_proj, up_proj, down_proj, num_tokens_per_expert,
            trace=False, trace_kwargs=None):
    from concourse.bass_utils import run_bass_kernel_spmd

    x = np.asarray(x)
    counts = np.asarray(num_tokens_per_expert).astype(np.int64)
    assert counts.shape == (NUM_EXPERTS,)
    starts = np.zeros(NUM_EXPERTS + 1, dtype=np.int64)
    np.cumsum(counts, out=starts[1:])

    # per-slot padded counts (max over cores) -> one SPMD program
    cmat = counts.reshape(N_CORES, EPC)
    padded = [int(cmat[:, j].max()) for j in range(EPC)]
    offs = np.zeros(EPC + 1, dtype=np.int64)
    np.cumsum(np.asarray(padded), out=offs[1:])

    nc, ptot = _get_program(padded)
    assert ptot == int(offs[-1])
    segs, _ = _layout(padded)

    gb = np.asarray(gate_proj).astype(BF16_NP)
    ub = np.asarray(up_proj).astype(BF16_NP)
    db = np.asarray(down_proj).astype(BF16_NP)

    in_maps = []
    for c in range(N_CORES):
        xtc = np.zeros((HID, ptot), dtype=BF16_NP)
        for j in range(EPC):
            e = c * EPC + j
            cnt = int(counts[e])
            if cnt:
                xtc[:, int(offs[j]):int(offs[j]) + cnt] = \
                    x[int(starts[e]):int(starts[e]) + cnt].astype(BF16_NP).T
        in_maps.append({
            "xt2": _swizzle_x(xtc, segs),
            "gw2": _swizzle_gu(gb[c * EPC:(c + 1) * EPC]),
            "uw2": _swizzle_gu(ub[c * EPC:(c + 1) * EPC]),
            "dw2": _swizzle_d(db[c * EPC:(c + 1) * EPC]),
        })

    res = run_bass_kernel_spmd(nc, in_maps, list(range(N_CORES)),
                               trace=trace, **(trace_kwargs or {}))

    out = np.empty((int(starts[-1]), HID), dtype=np.float32)
    for c in range(N_CORES):
        ytc = res.results[c]["yt"]
        for j in range(EPC):
            e = c * EPC + j
            cnt = int(counts[e])
            if cnt:
                out[int(starts[e]):int(starts[e]) + cnt] = \
                    ytc[:, int(offs[j]):int(offs[j]) + cnt].T.astype(np.float32)
    return out, res


def kernel(x, gate_proj, up_proj, down_proj, num_tokens_per_expert):
    out, _ = _invoke(x, gate_proj, up_proj, down_proj, num_tokens_per_expert)
    return out


# revision 11
# speedup vs baseline: 1.0414x; 1.0314x over previous
"""Grouped-experts MoE (SwiGLU) Bass kernel for Trainium2, 8 NeuronCores.

Expert-parallel: core c owns experts [8c, 8c+8). Tokens are pre-grouped by
expert in the input, so routing is host-side slicing. All device matmuls run
in transposed-token space so every operand streams in its natural layout:

  gateT[i, t] = sum_k G[k, i] * xT[k, t]      (lhsT = G tile, rhs = xT tile)
  hT = silu(gateT) * upT                       (elementwise, [inter, tok])
  outT[m, t] = sum_ki D[ki, m] * hT[ki, t]     (lhsT = D tile, rhs = hT tile)

v2 layout strategy: the host pre-swizzles every operand into the exact
column order the device consumes, so each SBUF tile is filled by ONE large
contiguous DMA:
  - x:   one tile per (segment, chunk): [128, 16k x csz]
  - G/U: one tile per (segment, i-pair): [128, 16k x 2i x 128]
  - D:   one tile per (segment, m-quartet): [128, 6ki x 4m x 128]
Fine tile granularity means tiles die early in each phase, freeing pool
slots so next-segment DMAs prefetch ~a full phase ahead (no PE stalls at
segment boundaries). Output is written bf16 (halves store traffic).
A burst of dummy matmuls on zeroed SBUF warms the PE HAM clock-gate during
the initial DMA ramp so real matmuls never run at the cold 1.2 GHz rate.
"""

import numpy as np
import ml_dtypes

NUM_EXPERTS = 64
HID = 2048
INTER = 768
N_CORES = 8
EPC = NUM_EXPERTS // N_CORES  # experts per core
KT = HID // 128    # 16 k-tiles over hidden
IT = INTER // 128  # 6 tiles over intermediate
NPAIR = IT // 2    # 3 i-pairs (gate/up weight tile granularity)
NQUAD = KT // 4    # 4 m-quartets (down weight tile granularity)
CHUNK = 512        # max moving-operand free dim per matmul
N_WARM = 28        # dummy matmuls to warm the PE clock gate

BF16_NP = ml_dtypes.bfloat16

_cache = {}


def _chunks(p):
    """Balanced split into ceil(p/CHUNK) near-equal chunks."""
    if p <= 0:
        return []
    nch = -(-p // CHUNK)
    base, rem = divmod(p, nch)
    out = []
    n0 = 0
    for i in range(nch):
        sz = base + (1 if i < rem else 0)
        out.append((n0, sz))
        n0 += sz
    return out


def _layout(padded):
    """Per-segment chunk lists and x-block column offsets in xt2."""
    segs = []
    xoff = 0
    col0 = 0
    for j, p in enumerate(padded):
        ch = _chunks(p)
        xoffs = []
        for (n0, csz) in ch:
            xoffs.append(xoff)
            xoff += KT * csz
        segs.append((j, col0, ch, xoffs))
        col0 += p
    return segs, xoff


def _build(padded):
    import concourse.bacc as bacc
    import concourse.mybir as mybir
    import concourse.tile as tile
    from concourse.tile import add_dep_helper

    BF16 = mybir.dt.bfloat16
    F32 = mybir.dt.float32
    SILU = mybir.ActivationFunctionType.Silu

    ptot = int(sum(padded))
    segs, xcols = _layout(padded)
    max_csz = max(csz for (_, _, ch, _) in segs for (_, csz) in ch)
    max_nch = max(len(ch) for (_, _, ch, _) in segs)
    h_live = IT * max_nch  # all h tiles of one segment live through its down phase

    nc = bacc.Bacc("TRN2", target_bir_lowering=False, debug=False,
                   num_devices=N_CORES)

    xt2 = nc.dram_tensor("xt2", [128, xcols], BF16, kind="ExternalInput")
    gw2 = nc.dram_tensor("gw2", [128, EPC * NPAIR * KT * 256], BF16,
                         kind="ExternalInput")
    uw2 = nc.dram_tensor("uw2", [128, EPC * NPAIR * KT * 256], BF16,
                         kind="ExternalInput")
    dw2 = nc.dram_tensor("dw2", [128, EPC * NQUAD * IT * 512], BF16,
                         kind="ExternalInput")
    yt = nc.dram_tensor("yt", [HID, ptot], BF16, kind="ExternalOutput")

    WCOLS = KT * 256   # columns per (slot, pair) G/U block
    DCOLS = IT * 512   # columns per (slot, quartet) D block

    with tile.TileContext(nc) as tc:
        with (
            tc.tile_pool(name="dmp", bufs=1) as dmp,
            tc.tile_pool(name="xp", bufs=max_nch + 2) as xp,
            tc.tile_pool(name="wp", bufs=8) as wp,
            tc.tile_pool(name="dp", bufs=3) as dp,
            tc.tile_pool(name="hp", bufs=h_live + 2) as hp,
            tc.tile_pool(name="sp", bufs=3) as sp,
            tc.tile_pool(name="op", bufs=4) as op,
            tc.tile_pool(name="psg", bufs=3, space="PSUM") as psg,
            tc.tile_pool(name="psu", bufs=3, space="PSUM") as psu,
            tc.tile_pool(name="psd", bufs=2, space="PSUM") as psd,
        ):
            # Chain every PE accumulation group onto the previous one
            # (order-only edges): the scheduler otherwise hoists groups whose
            # DMA it models as ready; a mispredict blocks the in-order PE
            # behind a waiting matmul while ready work sits queued.
            prev_mm = [None]

            def group(mms):
                first = None
                for (out_ap, lhsT, rhs, start, stop) in mms:
                    ins = nc.tensor.matmul(out_ap, lhsT, rhs,
                                           start=start, stop=stop)
                    if first is None:
                        first = ins
                        if prev_mm[0] is not None:
                            add_dep_helper(ins.ins, prev_mm[0].ins, sync=False,
                                           reason="pe-order")
                prev_mm[0] = ins

            # ---- PE warm-up: dummy matmuls on zeroed SBUF during DMA ramp
            dmy = dmp.tile([128, 640], BF16, tag="dmy")
            nc.vector.memset(dmy[:], 0.0)
            pwarm = psd.tile([128, 512], F32, tag="pd")
            for _ in range(N_WARM):
                group([(pwarm[:], dmy[:, 0:128], dmy[:, 128:640], True, True)])

            for si, (slot, col0, ch, xoffs) in enumerate(segs):
                nch = len(ch)
                halves = 2 if si == 0 else 1  # split first loads -> faster ramp

                # ---- loads, emitted in consumption order on the SP ring:
                # x(ch0), G(p0), U(p0), then remaining x chunks, then the
                # later weight pairs, then D. The HWDGE ring is FIFO, so
                # emission order is arrival order.
                def _load_w(dst, src, base, nh):
                    wstep = WCOLS // nh
                    for hh in range(nh):
                        nc.sync.dma_start(
                            dst[:, hh * wstep:(hh + 1) * wstep],
                            src[:, base + hh * wstep:base + (hh + 1) * wstep])

                xts, gt, ut = [], [], []
                for ci, (n0, csz) in enumerate(ch):
                    t = xp.tile([128, KT * max_csz], BF16, tag="x")
                    nh = halves if ci == 0 else 1
                    step = KT * csz // nh
                    for hh in range(nh):
                        nc.sync.dma_start(
                            t[:, hh * step:(hh + 1) * step],
                            xt2[:, xoffs[ci] + hh * step:
                                 xoffs[ci] + (hh + 1) * step])
                    xts.append(t)
                    if ci == 0:
                        base = slot * NPAIR * WCOLS
                        g = wp.tile([128, WCOLS], BF16, tag="w")
                        _load_w(g, gw2, base, halves)
                        gt.append(g)
                        u = wp.tile([128, WCOLS], BF16, tag="w")
                        _load_w(u, uw2, base, halves)
                        ut.append(u)
                for p in range(1, NPAIR):
                    base = (slot * NPAIR + p) * WCOLS
                    g = wp.tile([128, WCOLS], BF16, tag="w")
                    _load_w(g, gw2, base, 1)
                    gt.append(g)
                    u = wp.tile([128, WCOLS], BF16, tag="w")
                    _load_w(u, uw2, base, 1)
                    ut.append(u)
                dt = []
                for q in range(NQUAD):
                    d = dp.tile([128, DCOLS], BF16, tag="d")
                    base = (slot * NQUAD + q) * DCOLS
                    nc.sync.dma_start(d[:], dw2[:, base:base + DCOLS])
                    dt.append(d)

                # ---- gate/up phase ----
                h = {}
                for i in range(IT):
                    p, io = divmod(i, 2)
                    for ci, (n0, csz) in enumerate(ch):
                        pg = psg.tile([128, csz], F32, tag="pg")
                        group([(pg[:],
                                gt[p][:, k * 256 + io * 128:
                                      k * 256 + io * 128 + 128],
                                xts[ci][:, k * csz:(k + 1) * csz],
                                k == 0, k == KT - 1) for k in range(KT)])
                        pu = psu.tile([128, csz], F32, tag="pu")
                        group([(pu[:],
                                ut[p][:, k * 256 + io * 128:
                                      k * 256 + io * 128 + 128],
                                xts[ci][:, k * csz:(k + 1) * csz],
                                k == 0, k == KT - 1) for k in range(KT)])
                        st = sp.tile([128, csz], F32, tag="s")
                        nc.scalar.activation(st[:], pg[:], SILU)
                        ht = hp.tile([128, csz], BF16, tag="h")
                        nc.vector.tensor_mul(ht[:], st[:], pu[:])
                        h[(i, ci)] = ht

                # ---- down phase ----
                # One op tile and ONE out-DMA per m (all chunks batched),
                # on the ACT HWDGE ring: few big stores, not many tiny
                # SWDGE ones (whose ~2us/DMA overhead backed up the op pool
                # and stalled the PE through the cast->psum chain).
                seg_len = int(sum(csz for (_, csz) in ch))
                for m in range(KT):
                    q, mq = divmod(m, 4)
                    ot = op.tile([128, max_csz * len(ch)], BF16, tag="o")
                    for ci, (n0, csz) in enumerate(ch):
                        pd = psd.tile([128, csz], F32, tag="pd")
                        group([(pd[:],
                                dt[q][:, ki * 512 + mq * 128:
                                      ki * 512 + mq * 128 + 128],
                                h[(ki, ci)][:],
                                ki == 0, ki == IT - 1) for ki in range(IT)])
                        nc.vector.tensor_copy(ot[:, n0:n0 + csz], pd[:])
                    nc.scalar.dma_start(
                        yt[m * 128:(m + 1) * 128, col0:col0 + seg_len],
                        ot[:, :seg_len])

    nc.compile()
    return nc, ptot


def _get_program(padded):
    key = tuple(padded)
    if key not in _cache:
        _cache[key] = _build(padded)
    return _cache[key]


def _swizzle_x(xtc, segs):
    """[HID, ptot] -> [128, 16k x csz] blocks in device consumption order."""
    blocks = []
    for (_, col0, ch, _) in segs:
        for (n0, csz) in ch:
            blk = xtc[:, col0 + n0:col0 + n0 + csz]
            blocks.append(blk.reshape(KT, 128, csz).transpose(1, 0, 2)
                          .reshape(128, KT * csz))
    return np.ascontiguousarray(np.concatenate(blocks, axis=1))


def _swizzle_gu(w):
    """[EPC, HID, INTER] -> [128, EPC*3pair*(16k x 2io x 128)]."""
    # [e, k*128+p, (2pr+io)*128+c] -> cols (e, pr, k, io, c)
    arr = w.reshape(EPC, KT, 128, NPAIR, 2, 128)
    arr = arr.transpose(2, 0, 3, 1, 4, 5)  # [p, e, pr, k, io, c]
    return np.ascontiguousarray(arr.reshape(128, EPC * NPAIR * KT * 256))


def _swizzle_d(w):
    """[EPC, INTER, HID] -> [128, EPC*4quad*(6ki x 4mq x 128)]."""
    # [e, ki*128+p, (4q+mq)*128+c] -> cols (e, q, ki, mq, c)
    arr = w.reshape(EPC, IT, 128, NQUAD, 4, 128)
    arr = arr.transpose(2, 0, 3, 1, 4, 5)  # [p, e, q, ki, mq, c]
    return np.ascontiguousarray(arr.reshape(128, EPC * NQUAD * IT * 512))


def _invoke(x, gate_proj, up_proj, down_proj, num_tokens_per_expert,
            trace=False, trace_kwargs=None):
    from concourse.bass_utils import run_bass_kernel_spmd

    x = np.asarray(x)
    counts = np.asarray(num_tokens_per_expert).astype(np.int64)
    assert counts.shape == (NUM_EXPERTS,)
    starts = np.zeros(NUM_EXPERTS + 1, dtype=np.int64)
    np.cumsum(counts, out=starts[1:])

    # per-slot padded counts (max over cores) -> one SPMD program
    cmat = counts.reshape(N_CORES, EPC)
    padded = [int(cmat[:, j].max()) for j in range(EPC)]
    offs = np.zeros(EPC + 1, dtype=np.int64)
    np.cumsum(np.asarray(padded), out=offs[1:])

    nc, ptot = _get_program(padded)
    assert ptot == int(offs[-1])
    segs, _ = _layout(padded)

    gb = np.asarray(gate_proj).astype(BF16_NP)
    ub = np.asarray(up_proj).astype(BF16_NP)
    db = np.asarray(down_proj).astype(BF16_NP)

    in_maps = []
    for c in range(N_CORES):
        xtc = np.zeros((HID, ptot), dtype=BF16_NP)
        for j in range(EPC):
            e = c * EPC + j
            cnt = int(counts[e])
            if cnt:
                xtc[:, int(offs[j]):int(offs[j]) + cnt] = \
                    x[int(starts[e]):int(starts[e]) + cnt].astype(BF16_NP).T
        in_maps.append({
            "xt2": _swizzle_x(xtc, segs),
            "gw2": _swizzle_gu(gb[c * EPC:(c + 1) * EPC]),
            "uw2": _swizzle_gu(ub[c * EPC:(c + 1) * EPC]),
            "dw2": _swizzle_d(db[c * EPC:(c + 1) * EPC]),
        })

    res = run_bass_kernel_spmd(nc, in_maps, list(range(N_CORES)),
                               trace=trace, **(trace_kwargs or {}))

    out = np.empty((int(starts[-1]), HID), dtype=np.float32)
    for c in range(N_CORES):
        ytc = res.results[c]["yt"]
        for j in range(EPC):
            e = c * EPC + j
            cnt = int(counts[e])
            if cnt:
                out[int(starts[e]):int(starts[e]) + cnt] = \
                    ytc[:, int(offs[j]):int(offs[j]) + cnt].T.astype(np.float32)
    return out, res


def kernel(x, gate_proj, up_proj, down_proj, num_tokens_per_expert):
    out, _ = _invoke(x, gate_proj, up_proj, down_proj, num_tokens_per_expert)
    return out
